# revision 24
# baseline (speedup 1.0000x reference)
"""Correlation-cycle (Chamfer) loss kernel for Trainium2, 8 NeuronCores.

reference:  P[b,i,j] = ||x_i||^2 + ||y_j||^2 - 2 x_i.y_j   (x=corr_pred, y=corr_target)
            out = (mean_{b,j} min_i clip(P,0,100) + mean_{b,i} min_j clip(P,0,100)) / B

Sharding: B=4 batches x 2 i-halves -> 8 cores. Each core owns an x-half
(2048 rows) and the full y (4096 rows) of one batch.

Scheme "hybrid" (default): ONE matmul orientation [i x j]; each PSUM group
[128 x 2048] (= -2*z, bf16 inputs / fp32 accumulate) is consumed by exactly
two fused DVE ops:
  tensor_tensor_reduce: u_bf16 = psum + y2bc ( = y2_j - 2 z_ij );
                        rowacc[:,ic] = min(rowacc[:,ic], min_j u)   (row mins)
  scalar_tensor_tensor: colacc = min(colacc, u + x2_i)             (col mins,
                        colacc accumulates the full P over i-chunks per lane)
Host: min over lanes/cores (+x2_i for rows), clip(0,100) -- clip commutes
with min -- then means.  No ACT/Pool usage; minimal instruction count (the
axon-virtualized NeuronCores are instruction-dispatch-bound at ~2us/inst,
so half the matmuls + 2 DVE ops per group beat any multi-engine split).

Scheme "pf": two orientations with a D/A engine split (kept for A/B).
"""

import numpy as np
import ml_dtypes

import concourse.bass as bass
import concourse.mybir as mybir
import concourse.tile as tile
from concourse import bacc
from concourse.bass_utils import run_bass_kernel_spmd

BF16 = ml_dtypes.bfloat16
F32 = np.float32

B, N, D = 4, 4096, 128
NCORES = 8
NI = N // 2          # per-core i range (half a batch)
NJ = N               # full j range
GW = 2048            # psum group width (4 banks)
MMW = 512            # matmul moving width (1 bank)
BIG = 1.0e38         # accumulator init (min identity; fits bf16)

AluOp = mybir.AluOpType
ActFn = mybir.ActivationFunctionType

# pf-scheme routing pattern (D = DVE-direct fp32, A = ACT->DVE bf16)
PAT1 = ['D', 'A', 'A', 'A'] * 4
PAT2 = PAT1 + PAT1


def build(ni=NI, nj=NJ, gw=GW, reps=1, scheme="ttr"):
    if scheme == "pf":
        return build_pf(ni, nj, min(gw, 1024), reps)
    if scheme == "v3":
        return build_v3(ni, nj, gw, reps)
    if scheme.startswith("ttr"):
        return build_ttr(ni, nj, gw, reps,
                         use_rank1="nor1" not in scheme,
                         use_ttr="nottr" not in scheme)
    n_ic = ni // 128
    n_jg = nj // gw

    nc = bacc.Bacc("TRN2", target_bir_lowering=False, debug=False,
                   enable_asserts=False, num_devices=NCORES)
    f32 = mybir.dt.float32
    bf16 = mybir.dt.bfloat16

    xT_d = nc.dram_tensor("xT", [128, ni], bf16, kind="ExternalInput")
    m2yT_d = nc.dram_tensor("m2yT", [128, nj], bf16, kind="ExternalInput")
    x2c_d = nc.dram_tensor("x2c", [128, n_ic], f32, kind="ExternalInput")
    y2bc_d = nc.dram_tensor("y2bc", [128, nj], bf16, kind="ExternalInput")
    colB_d = nc.dram_tensor("colB", [128, nj], bf16, kind="ExternalOutput")
    rowR_d = nc.dram_tensor("rowR", [128, n_ic * n_jg], f32, kind="ExternalOutput")

    with tile.TileContext(nc) as tc:
        with (
            tc.tile_pool(name="persist", bufs=1) as persist,
            tc.tile_pool(name="psum", bufs=2, space="PSUM") as psum_pool,
            tc.tile_pool(name="u", bufs=3) as upool,
        ):
            xT = persist.tile([128, ni], bf16, name="xT")
            m2yT = persist.tile([128, nj], bf16, name="m2yT")
            x2c = persist.tile([128, n_ic], f32, name="x2c")
            y2bc = persist.tile([128, nj], bf16, name="y2bc")
            colB = persist.tile([128, nj], bf16, name="colB")
            rowR = persist.tile([128, n_ic * n_jg], f32, name="rowR")

            nc.sync.dma_start(out=xT[:, :], in_=xT_d[:, :])
            ck = min(2048, nj)
            for c0 in range(0, nj, ck):
                nc.sync.dma_start(out=m2yT[:, c0:c0 + ck], in_=m2yT_d[:, c0:c0 + ck])
                nc.sync.dma_start(out=y2bc[:, c0:c0 + ck], in_=y2bc_d[:, c0:c0 + ck])
            nc.sync.dma_start(out=x2c[:, :], in_=x2c_d[:, :])
            nc.vector.memset(colB[:, :], BIG)

            def emit_body():
                for ic in range(n_ic):
                    for jg in range(n_jg):
                        sl = slice(jg * gw, (jg + 1) * gw)
                        psum = psum_pool.tile([128, gw], f32, tag="ps", name="ps")
                        for q in range(gw // MMW):
                            j0 = jg * gw + q * MMW
                            nc.tensor.matmul(
                                psum[:, q * MMW:(q + 1) * MMW],
                                xT[:, ic * 128:(ic + 1) * 128],
                                m2yT[:, j0:j0 + MMW])
                        u = upool.tile([128, gw], bf16, tag="u", name="u")
                        nc.vector.tensor_tensor(
                            u[:, :], psum[:, :], y2bc[:, sl], AluOp.add)
                        k = ic * n_jg + jg
                        nc.vector.tensor_reduce(
                            rowR[:, k:k + 1], u[:, :],
                            mybir.AxisListType.X, AluOp.min)
                        nc.vector.scalar_tensor_tensor(
                            colB[:, sl], u[:, :], x2c[:, ic:ic + 1],
                            colB[:, sl], AluOp.add, AluOp.min)

            if reps > 1:
                with tc.For_i(0, reps, 1,
                              hint_engines=(mybir.EngineType.PE,
                                            mybir.EngineType.DVE)):
                    emit_body()
            else:
                emit_body()

            for c0 in range(0, nj, ck):
                nc.sync.dma_start(out=colB_d[:, c0:c0 + ck], in_=colB[:, c0:c0 + ck])
            nc.sync.dma_start(out=rowR_d[:, :], in_=rowR[:, :])

    nc.compile()
    return nc


def build_ttr(ni=NI, nj=NJ, gw=GW, reps=1, use_rank1=True, use_ttr=True):
    """One-orientation scheme, 2 DVE ops per [128 x gw] psum group:

      PE:  psum = -2*z + x2_i   (4 data MMs + 4 rank-1 MMs per group;
           rank-1: stationary row0 = x2 chunk, moving row0 = ones)
      DVE: tensor_tensor_reduce: u_bf16 = psum + y2bc  (full P),
           rowR[:,k] = min_j u   -- fused conversion + row-min (1x pass)
      DVE: tensor_tensor:        colB = min(colB, u)   (bf16 2x pass)

    Host: row mins = min over jg of rowR; col mins = min over lanes/cores
    of colB; clip(0,100) commutes with min; then means.
    """
    n_ic = ni // 128
    n_jg = nj // gw

    nc = bacc.Bacc("TRN2", target_bir_lowering=False, debug=False,
                   enable_asserts=False, num_devices=NCORES)
    f32 = mybir.dt.float32
    bf16 = mybir.dt.bfloat16

    xT_d = nc.dram_tensor("xT", [128, ni], bf16, kind="ExternalInput")
    m2yT_d = nc.dram_tensor("m2yT", [128, nj], bf16, kind="ExternalInput")
    if use_rank1:
        x2r_d = nc.dram_tensor("x2r", [128, n_ic * 128], bf16,
                               kind="ExternalInput")
        ones_d = nc.dram_tensor("ones", [128, MMW], bf16, kind="ExternalInput")
    else:
        x2c_d = nc.dram_tensor("x2c", [128, n_ic], f32, kind="ExternalInput")
    y2bc_d = nc.dram_tensor("y2bc", [128, nj], bf16, kind="ExternalInput")
    colB_d = nc.dram_tensor("colB", [128, nj], bf16, kind="ExternalOutput")
    rowR_d = nc.dram_tensor("rowR", [128, n_ic * n_jg], f32, kind="ExternalOutput")

    with tile.TileContext(nc) as tc:
        with (
            tc.tile_pool(name="persist", bufs=1) as persist,
            tc.tile_pool(name="psum", bufs=2, space="PSUM") as psum_pool,
            tc.tile_pool(name="u", bufs=3) as upool,
        ):
            xT = persist.tile([128, ni], bf16, name="xT")
            m2yT = persist.tile([128, nj], bf16, name="m2yT")
            if use_rank1:
                x2r = persist.tile([128, n_ic * 128], bf16, name="x2r")
                ones = persist.tile([128, MMW], bf16, name="ones")
            else:
                x2c = persist.tile([128, n_ic], f32, name="x2c")
            y2bc = persist.tile([128, nj], bf16, name="y2bc")
            colB = persist.tile([128, nj], bf16, name="colB")
            rowR = persist.tile([128, n_ic * n_jg], f32, name="rowR")

            nc.sync.dma_start(out=xT[:, :], in_=xT_d[:, :])
            if use_rank1:
                nc.sync.dma_start(out=x2r[:, :], in_=x2r_d[:, :])
                nc.sync.dma_start(out=ones[:, :], in_=ones_d[:, :])
            else:
                nc.sync.dma_start(out=x2c[:, :], in_=x2c_d[:, :])
            ck = min(2048, nj)
            for c0 in range(0, nj, ck):
                nc.sync.dma_start(out=m2yT[:, c0:c0 + ck], in_=m2yT_d[:, c0:c0 + ck])
                nc.sync.dma_start(out=y2bc[:, c0:c0 + ck], in_=y2bc_d[:, c0:c0 + ck])
            nc.vector.memset(colB[:, :], BIG)

            def emit_body():
                for ic in range(n_ic):
                    for jg in range(n_jg):
                        sl = slice(jg * gw, (jg + 1) * gw)
                        psum = psum_pool.tile([128, gw], f32, tag="ps", name="ps")
                        for q in range(gw // MMW):
                            j0 = jg * gw + q * MMW
                            nc.tensor.matmul(
                                psum[:, q * MMW:(q + 1) * MMW],
                                xT[:, ic * 128:(ic + 1) * 128],
                                m2yT[:, j0:j0 + MMW],
                                start=True, stop=not use_rank1)
                        if use_rank1:
                            for q in range(gw // MMW):
                                nc.tensor.matmul(
                                    psum[:, q * MMW:(q + 1) * MMW],
                                    x2r[:, ic * 128:(ic + 1) * 128],
                                    ones[:, :],
                                    start=False, stop=True)
                        u = upool.tile([128, gw], bf16, tag="u", name="u")
                        k = ic * n_jg + jg
                        if use_ttr:
                            nc.vector.tensor_tensor_reduce(
                                u[:, :], psum[:, :], y2bc[:, sl],
                                1.0, BIG, AluOp.add, AluOp.min,
                                rowR[:, k:k + 1])
                        else:
                            nc.vector.tensor_tensor(
                                u[:, :], psum[:, :], y2bc[:, sl], AluOp.add)
                            nc.vector.tensor_reduce(
                                rowR[:, k:k + 1], u[:, :],
                                mybir.AxisListType.X, AluOp.min)
                        if use_rank1:
                            nc.vector.tensor_tensor(
                                colB[:, sl], u[:, :], colB[:, sl], AluOp.min)
                        else:
                            nc.vector.scalar_tensor_tensor(
                                colB[:, sl], u[:, :], x2c[:, ic:ic + 1],
                                colB[:, sl], AluOp.add, AluOp.min)

            if reps > 1:
                with tc.For_i(0, reps, 1,
                              hint_engines=(mybir.EngineType.PE,
                                            mybir.EngineType.DVE)):
                    emit_body()
            else:
                emit_body()

            for c0 in range(0, nj, ck):
                nc.sync.dma_start(out=colB_d[:, c0:c0 + ck], in_=colB[:, c0:c0 + ck])
            nc.sync.dma_start(out=rowR_d[:, :], in_=rowR[:, :])

    nc.compile()
    return nc


TW = 512  # row-min tree stop width (host finishes the reduction)


def build_v3(ni=NI, nj=NJ, gw=GW, reps=1):
    """Measured-rate design (TTR unusable on this HW):

      PE:  psum = -2z + x2_i + y2_j   (4 data MMs + 4 rank-2 MMs;
           rank-2 stationary row0 = x2 chunk, row1 = ones;
           rank-2 moving row0 = ones, row1 = y2)
      ACT: u = bf16(psum)        1x @1.2GHz, own engine   (~1966ns/group)
      DVE: colB = min(colB, u)   bf16 SBUF 2x             (~1226ns/group)
      DVE: row-min tree of TT-mins at 2x down to TW, then one segmented
           tensor_reduce [128, n_ic, TW] -> [128, n_ic] at the end.

    Host: rowR/colB hold full-P mins; clip+mean only.
    """
    n_ic = ni // 128
    n_jg = nj // gw

    nc = bacc.Bacc("TRN2", target_bir_lowering=False, debug=False,
                   enable_asserts=False, num_devices=NCORES)
    f32 = mybir.dt.float32
    bf16 = mybir.dt.bfloat16

    xT_d = nc.dram_tensor("xT", [128, ni], bf16, kind="ExternalInput")
    m2yT_d = nc.dram_tensor("m2yT", [128, nj], bf16, kind="ExternalInput")
    r2s_d = nc.dram_tensor("r2s", [128, n_ic * 128], bf16, kind="ExternalInput")
    r2m_d = nc.dram_tensor("r2m", [128, nj], bf16, kind="ExternalInput")
    colB_d = nc.dram_tensor("colB", [128, nj], bf16, kind="ExternalOutput")
    rowA_d = nc.dram_tensor("rowA", [128, n_ic * TW], bf16, kind="ExternalOutput")

    with tile.TileContext(nc) as tc:
        with (
            tc.tile_pool(name="persist", bufs=1) as persist,
            tc.tile_pool(name="psum", bufs=2, space="PSUM") as psum_pool,
            tc.tile_pool(name="u", bufs=4) as upool,
            tc.tile_pool(name="tree", bufs=2) as tpool,
        ):
            xT = persist.tile([128, ni], bf16, name="xT")
            m2yT = persist.tile([128, nj], bf16, name="m2yT")
            r2s = persist.tile([128, n_ic * 128], bf16, name="r2s")
            r2m = persist.tile([128, nj], bf16, name="r2m")
            colB = persist.tile([128, nj], bf16, name="colB")
            rowacc = persist.tile([128, n_ic * TW], bf16, name="rowacc")

            # first-group inputs first so compute starts ASAP
            fk = min(512, ni)
            nc.sync.dma_start(out=xT[:, :fk], in_=xT_d[:, :fk])
            nc.sync.dma_start(out=r2s[:, :fk], in_=r2s_d[:, :fk])
            fj = min(gw, nj)
            nc.sync.dma_start(out=m2yT[:, :fj], in_=m2yT_d[:, :fj])
            nc.sync.dma_start(out=r2m[:, :fj], in_=r2m_d[:, :fj])
            if fk < ni:
                nc.sync.dma_start(out=xT[:, fk:], in_=xT_d[:, fk:])
                nc.sync.dma_start(out=r2s[:, fk:], in_=r2s_d[:, fk:])
            for c0 in range(fj, nj, gw):
                nc.sync.dma_start(out=m2yT[:, c0:c0 + gw], in_=m2yT_d[:, c0:c0 + gw])
                nc.sync.dma_start(out=r2m[:, c0:c0 + gw], in_=r2m_d[:, c0:c0 + gw])

            def emit_body():
                for ic in range(n_ic):
                    st = slice(ic * 128, (ic + 1) * 128)
                    u = upool.tile([128, nj], bf16, tag="u", name="u")
                    for jg in range(n_jg):
                        sl = slice(jg * gw, (jg + 1) * gw)
                        psum = psum_pool.tile([128, gw], f32, tag="ps", name="ps")
                        for q in range(gw // MMW):
                            j0 = jg * gw + q * MMW
                            nc.tensor.matmul(
                                psum[:, q * MMW:(q + 1) * MMW],
                                xT[:, st], m2yT[:, j0:j0 + MMW],
                                start=True, stop=False)
                        for q in range(gw // MMW):
                            j0 = jg * gw + q * MMW
                            nc.tensor.matmul(
                                psum[:, q * MMW:(q + 1) * MMW],
                                r2s[:, st], r2m[:, j0:j0 + MMW],
                                start=False, stop=True)
                        nc.scalar.activation(u[:, sl], psum[:, :],
                                             ActFn.Identity, bias=0.0,
                                             scale=1.0)
                    # full-width col accumulate (2x, or 4x copy for ic 0)
                    if ic == 0:
                        nc.vector.tensor_copy(colB[:, :], u[:, :])
                    else:
                        nc.vector.tensor_tensor(
                            colB[:, :], u[:, :], colB[:, :], AluOp.min)
                    # row-min tree: nj -> TW in bf16 TT-mins (2x)
                    h = nj // 2
                    a = tpool.tile([128, h], bf16, tag="a", name="a")
                    nc.vector.tensor_tensor(
                        a[:, :], u[:, :h], u[:, h:], AluOp.min)
                    w = h
                    while w > TW * 2:
                        w //= 2
                        nc.vector.tensor_tensor(
                            a[:, :w], a[:, :w], a[:, w:2 * w], AluOp.min)
                    rsl = slice(ic * TW, (ic + 1) * TW)
                    nc.vector.tensor_tensor(
                        rowacc[:, rsl], a[:, :TW], a[:, TW:2 * TW], AluOp.min)
                    nc.sync.dma_start(out=rowA_d[:, rsl], in_=rowacc[:, rsl])

            if reps > 1:
                with tc.For_i(0, reps, 1,
                              hint_engines=(mybir.EngineType.PE,
                                            mybir.EngineType.DVE,
                                            mybir.EngineType.Activation)):
                    emit_body()
            else:
                emit_body()

            for c0 in range(0, nj, gw):
                nc.sync.dma_start(out=colB_d[:, c0:c0 + gw], in_=colB[:, c0:c0 + gw])

    nc.compile()
    return nc


def build_pf(ni, nj, gw, reps):
    """Two-orientation scheme with DVE/ACT split (fallback / A-B testing)."""
    n_ic = ni // 128
    n_jc = nj // 128
    pat1 = PAT1[:n_ic]
    pat2 = PAT2[:n_jc]
    paths = set(pat1) | set(pat2)

    nc = bacc.Bacc("TRN2", target_bir_lowering=False, debug=False,
                   enable_asserts=False, num_devices=NCORES)
    f32 = mybir.dt.float32
    bf16 = mybir.dt.bfloat16

    xT_d = nc.dram_tensor("xT", [128, ni], bf16, kind="ExternalInput")
    m2yT_d = nc.dram_tensor("m2yT", [128, nj], bf16, kind="ExternalInput")
    x2c_d = nc.dram_tensor("x2c", [128, n_ic], f32, kind="ExternalInput")
    y2c_d = nc.dram_tensor("y2c", [128, n_jc], f32, kind="ExternalInput")
    col_d, row_d = {}, {}
    for p in sorted(paths):
        dt = f32 if p == 'D' else bf16
        col_d[p] = nc.dram_tensor("col" + p, [128, nj], dt, kind="ExternalOutput")
        row_d[p] = nc.dram_tensor("row" + p, [128, ni], dt, kind="ExternalOutput")

    with tile.TileContext(nc) as tc:
        with (
            tc.tile_pool(name="persist", bufs=1) as persist,
            tc.tile_pool(name="psum", bufs=4, space="PSUM") as psum_pool,
            tc.tile_pool(name="u", bufs=6) as upool,
        ):
            xT = persist.tile([128, ni], bf16, name="xT")
            m2yT = persist.tile([128, nj], bf16, name="m2yT")
            x2c = persist.tile([128, n_ic], f32, name="x2c")
            y2c = persist.tile([128, n_jc], f32, name="y2c")
            col_s = {p: persist.tile([128, nj], f32 if p == 'D' else bf16,
                                     name="col" + p, tag="col" + p)
                     for p in sorted(paths)}
            row_s = {p: persist.tile([128, ni], f32 if p == 'D' else bf16,
                                     name="row" + p, tag="row" + p)
                     for p in sorted(paths)}

            ck = min(1024, ni, nj)
            for c0 in range(0, ni, ck):
                nc.sync.dma_start(out=xT[:, c0:c0 + ck], in_=xT_d[:, c0:c0 + ck])
            for c0 in range(0, nj, ck):
                nc.sync.dma_start(out=m2yT[:, c0:c0 + ck], in_=m2yT_d[:, c0:c0 + ck])
            nc.sync.dma_start(out=x2c[:, :], in_=x2c_d[:, :])
            nc.sync.dma_start(out=y2c[:, :], in_=y2c_d[:, :])

            def consume(path, psum, bias, accs, sl, first):
                acc = accs[path]
                if path == 'D':
                    if first:
                        nc.vector.tensor_scalar(
                            acc[:, sl], psum[:, :], bias, None, AluOp.add)
                    else:
                        nc.vector.scalar_tensor_tensor(
                            acc[:, sl], psum[:, :], bias, acc[:, sl],
                            AluOp.add, AluOp.min)
                    return
                u = upool.tile([128, psum.shape[1]], bf16, name="u", tag="u")
                nc.scalar.activation(u[:, :], psum[:, :], ActFn.Identity,
                                     bias=bias, scale=1.0)
                if first:
                    nc.vector.tensor_copy(acc[:, sl], u[:, :])
                else:
                    nc.vector.tensor_tensor(acc[:, sl], u[:, :], acc[:, sl],
                                            AluOp.min)

            def emit_body():
                for jg in range(nj // gw):
                    sl = slice(jg * gw, (jg + 1) * gw)
                    seen = set()
                    for ic in range(n_ic):
                        path = pat1[ic]
                        psum = psum_pool.tile([128, gw], f32, tag="ps", name="ps")
                        for q in range(gw // MMW):
                            j0 = jg * gw + q * MMW
                            nc.tensor.matmul(
                                psum[:, q * MMW:(q + 1) * MMW],
                                xT[:, ic * 128:(ic + 1) * 128],
                                m2yT[:, j0:j0 + MMW])
                        consume(path, psum, x2c[:, ic:ic + 1], col_s, sl,
                                path not in seen)
                        seen.add(path)
                gw2 = min(gw, ni)
                for ig in range(ni // gw2):
                    sl = slice(ig * gw2, (ig + 1) * gw2)
                    seen = set()
                    for jc in range(n_jc):
                        path = pat2[jc]
                        psum = psum_pool.tile([128, gw2], f32, tag="ps", name="ps")
                        for q in range(gw2 // MMW):
                            i0 = ig * gw2 + q * MMW
                            nc.tensor.matmul(
                                psum[:, q * MMW:(q + 1) * MMW],
                                m2yT[:, jc * 128:(jc + 1) * 128],
                                xT[:, i0:i0 + MMW])
                        consume(path, psum, y2c[:, jc:jc + 1], row_s, sl,
                                path not in seen)
                        seen.add(path)

            if reps > 1:
                with tc.For_i(0, reps, 1,
                              hint_engines=(mybir.EngineType.PE,
                                            mybir.EngineType.DVE,
                                            mybir.EngineType.Activation)):
                    emit_body()
            else:
                emit_body()

            for p in sorted(paths):
                nc.sync.dma_start(out=col_d[p][:, :], in_=col_s[p][:, :])
                nc.sync.dma_start(out=row_d[p][:, :], in_=row_s[p][:, :])

    nc.compile()
    return nc


def host_prep(x, y, scheme="hybrid"):
    """Per-core input maps. Core c: batch c//2, i-half c%2."""
    x = np.ascontiguousarray(np.asarray(x, F32))
    y = np.ascontiguousarray(np.asarray(y, F32))
    x16 = x.astype(BF16)
    y16 = y.astype(BF16)
    m2y16 = (y16.astype(F32) * -2.0).astype(BF16)          # exact in bf16
    x2 = (x16.astype(F32) ** 2).sum(-1)                    # [B, N]
    y2 = (y16.astype(F32) ** 2).sum(-1)
    in_maps = []
    for c in range(NCORES):
        b, h = divmod(c, 2)
        i0 = h * NI
        m = {
            "xT": np.ascontiguousarray(x16[b, i0:i0 + NI, :].T),
            "m2yT": np.ascontiguousarray(m2y16[b].T),
        }
        if scheme == "v3":
            r2s = np.zeros((128, NI), BF16)
            r2s[0, :] = x2[b, i0:i0 + NI].astype(BF16)
            r2s[1, :] = 1.0
            m["r2s"] = r2s
            r2m = np.zeros((128, N), BF16)
            r2m[0, :] = 1.0
            r2m[1, :] = y2[b].astype(BF16)
            m["r2m"] = r2m
        elif scheme.startswith("ttr"):
            if "nor1" in scheme:
                m["x2c"] = np.ascontiguousarray(
                    x2[b, i0:i0 + NI].reshape(NI // 128, 128).T)
            else:
                x2r = np.zeros((128, NI), BF16)
                x2r[0, :] = x2[b, i0:i0 + NI].astype(BF16)
                m["x2r"] = x2r
                ones = np.zeros((128, MMW), BF16)
                ones[0, :] = 1.0
                m["ones"] = ones
            m["y2bc"] = np.ascontiguousarray(
                np.broadcast_to(y2[b].astype(BF16), (128, N)))
        elif scheme == "hybrid":
            m["x2c"] = np.ascontiguousarray(
                x2[b, i0:i0 + NI].reshape(NI // 128, 128).T)
            m["y2bc"] = np.ascontiguousarray(
                np.broadcast_to(y2[b].astype(BF16), (128, N)))
        else:
            m["x2c"] = np.ascontiguousarray(
                x2[b, i0:i0 + NI].reshape(NI // 128, 128).T)
            m["y2c"] = np.ascontiguousarray(y2[b].reshape(N // 128, 128).T)
        in_maps.append(m)
    return in_maps, x2, y2


def combine(results, x2, y2, scheme="hybrid"):
    col_mins = np.empty((B, N), F32)
    row_mins = np.empty((B, N), F32)
    for b in range(B):
        cores = [results[2 * b], results[2 * b + 1]]
        if scheme == "v3":
            col = np.minimum.reduce([r["colB"].astype(F32).min(0) for r in cores])
            col_mins[b] = np.clip(col, 0.0, 100.0)
            for h, r in enumerate(cores):
                ra = r["rowA"].astype(F32)             # [128, n_ic*TW]
                row = ra.reshape(128, NI // 128, TW).min(axis=2).T.reshape(-1)
                i0 = h * NI
                row_mins[b, i0:i0 + NI] = np.clip(row, 0.0, 100.0)
        elif scheme.startswith("ttr"):
            # colB holds full P mins; clip commutes with min
            col = np.minimum.reduce([r["colB"].astype(F32).min(0) for r in cores])
            col_mins[b] = np.clip(col, 0.0, 100.0)
            for h, r in enumerate(cores):
                rr = r["rowR"]                         # [128, n_ic*n_jg]
                n_jg = N // GW
                rr = rr.reshape(128, NI // 128, n_jg).min(axis=2)
                row = rr.T.reshape(-1)                 # [NI], i = ic*128 + lane
                i0 = h * NI
                if "nor1" in scheme:                   # psum lacked x2
                    row = row + x2[b, i0:i0 + NI]
                row_mins[b, i0:i0 + NI] = np.clip(row, 0.0, 100.0)
        elif scheme == "hybrid":
            col = np.minimum.reduce([r["colB"].astype(F32).min(0) for r in cores])
            col_mins[b] = np.clip(col, 0.0, 100.0)
            for h, r in enumerate(cores):
                rr = r["rowR"]                         # [128, n_ic*n_jg]
                n_jg = N // GW
                rr = rr.reshape(128, NI // 128, n_jg).min(axis=2)
                row = rr.T.reshape(-1)                 # [NI], i = ic*128 + lane
                i0 = h * NI
                row_mins[b, i0:i0 + NI] = np.clip(
                    row + x2[b, i0:i0 + NI], 0.0, 100.0)
        else:
            col = np.minimum.reduce([
                np.minimum.reduce([r[k].astype(F32).min(0)
                                   for k in r if k.startswith("col")])
                for r in cores])
            col_mins[b] = np.clip(col + y2[b], 0.0, 100.0)
            for h, r in enumerate(cores):
                row = np.minimum.reduce([r[k].astype(F32).min(0)
                                         for k in r if k.startswith("row")])
                i0 = h * NI
                row_mins[b, i0:i0 + NI] = np.clip(
                    row + x2[b, i0:i0 + NI], 0.0, 100.0)
    out = (col_mins.mean(dtype=np.float64) + row_mins.mean(dtype=np.float64)) / B
    return np.asarray(out, dtype=F32)


_CACHE = {}
TRACE = False
LAST_RESULTS = None
SCHEME = "ttr"


def kernel(corr_pred, corr_target):
    global LAST_RESULTS
    key = ("nc", SCHEME)
    if key not in _CACHE:
        _CACHE[key] = build(scheme=SCHEME)
    nc = _CACHE[key]
    in_maps, x2, y2 = host_prep(corr_pred, corr_target, scheme=SCHEME)
    res = run_bass_kernel_spmd(nc, in_maps, core_ids=list(range(NCORES)),
                               trace=TRACE)
    LAST_RESULTS = res
    return combine(res.results, x2, y2, scheme=SCHEME)



# revision 26
# speedup vs baseline: 1.1913x; 1.1913x over previous
"""Correlation-cycle (Chamfer) loss kernel for Trainium2, 8 NeuronCores.

reference:  P[b,i,j] = ||x_i||^2 + ||y_j||^2 - 2 x_i.y_j   (x=corr_pred, y=corr_target)
            out = (mean_{b,j} min_i clip(P,0,100) + mean_{b,i} min_j clip(P,0,100)) / B

Sharding: B=4 batches x 2 i-halves -> 8 cores. Each core owns an x-half
(2048 rows) and the full y (4096 rows) of one batch.

Scheme "hybrid" (default): ONE matmul orientation [i x j]; each PSUM group
[128 x 2048] (= -2*z, bf16 inputs / fp32 accumulate) is consumed by exactly
two fused DVE ops:
  tensor_tensor_reduce: u_bf16 = psum + y2bc ( = y2_j - 2 z_ij );
                        rowacc[:,ic] = min(rowacc[:,ic], min_j u)   (row mins)
  scalar_tensor_tensor: colacc = min(colacc, u + x2_i)             (col mins,
                        colacc accumulates the full P over i-chunks per lane)
Host: min over lanes/cores (+x2_i for rows), clip(0,100) -- clip commutes
with min -- then means.  No ACT/Pool usage; minimal instruction count (the
axon-virtualized NeuronCores are instruction-dispatch-bound at ~2us/inst,
so half the matmuls + 2 DVE ops per group beat any multi-engine split).

Scheme "pf": two orientations with a D/A engine split (kept for A/B).
"""

import numpy as np
import ml_dtypes

import concourse.bass as bass
import concourse.mybir as mybir
import concourse.tile as tile
from concourse import bacc
from concourse.bass_utils import run_bass_kernel_spmd

BF16 = ml_dtypes.bfloat16
F32 = np.float32

B, N, D = 4, 4096, 128
NCORES = 8
NI = N // 2          # per-core i range (half a batch)
NJ = N               # full j range
GW = 2048            # psum group width (4 banks)
MMW = 512            # matmul moving width (1 bank)
BIG = 1.0e38         # accumulator init (min identity; fits bf16)

AluOp = mybir.AluOpType
ActFn = mybir.ActivationFunctionType

# pf-scheme routing pattern (D = DVE-direct fp32, A = ACT->DVE bf16)
PAT1 = ['D', 'A', 'A', 'A'] * 4
PAT2 = PAT1 + PAT1


def build(ni=NI, nj=NJ, gw=GW, reps=1, scheme="ttr"):
    if scheme == "pf":
        return build_pf(ni, nj, min(gw, 1024), reps)
    if scheme == "v3":
        return build_v3(ni, nj, gw, reps)
    if scheme.startswith("ttr"):
        return build_ttr(ni, nj, gw, reps,
                         use_rank1="nor1" not in scheme,
                         use_ttr="nottr" not in scheme)
    n_ic = ni // 128
    n_jg = nj // gw

    nc = bacc.Bacc("TRN2", target_bir_lowering=False, debug=False,
                   enable_asserts=False, num_devices=NCORES)
    f32 = mybir.dt.float32
    bf16 = mybir.dt.bfloat16

    xT_d = nc.dram_tensor("xT", [128, ni], bf16, kind="ExternalInput")
    m2yT_d = nc.dram_tensor("m2yT", [128, nj], bf16, kind="ExternalInput")
    x2c_d = nc.dram_tensor("x2c", [128, n_ic], f32, kind="ExternalInput")
    y2bc_d = nc.dram_tensor("y2bc", [128, nj], bf16, kind="ExternalInput")
    colB_d = nc.dram_tensor("colB", [128, nj], bf16, kind="ExternalOutput")
    rowR_d = nc.dram_tensor("rowR", [128, n_ic * n_jg], f32, kind="ExternalOutput")

    with tile.TileContext(nc) as tc:
        with (
            tc.tile_pool(name="persist", bufs=1) as persist,
            tc.tile_pool(name="psum", bufs=2, space="PSUM") as psum_pool,
            tc.tile_pool(name="u", bufs=3) as upool,
        ):
            xT = persist.tile([128, ni], bf16, name="xT")
            m2yT = persist.tile([128, nj], bf16, name="m2yT")
            x2c = persist.tile([128, n_ic], f32, name="x2c")
            y2bc = persist.tile([128, nj], bf16, name="y2bc")
            colB = persist.tile([128, nj], bf16, name="colB")
            rowR = persist.tile([128, n_ic * n_jg], f32, name="rowR")

            nc.sync.dma_start(out=xT[:, :], in_=xT_d[:, :])
            ck = min(2048, nj)
            for c0 in range(0, nj, ck):
                nc.sync.dma_start(out=m2yT[:, c0:c0 + ck], in_=m2yT_d[:, c0:c0 + ck])
                nc.sync.dma_start(out=y2bc[:, c0:c0 + ck], in_=y2bc_d[:, c0:c0 + ck])
            nc.sync.dma_start(out=x2c[:, :], in_=x2c_d[:, :])
            nc.vector.memset(colB[:, :], BIG)

            def emit_body():
                for ic in range(n_ic):
                    for jg in range(n_jg):
                        sl = slice(jg * gw, (jg + 1) * gw)
                        psum = psum_pool.tile([128, gw], f32, tag="ps", name="ps")
                        for q in range(gw // MMW):
                            j0 = jg * gw + q * MMW
                            nc.tensor.matmul(
                                psum[:, q * MMW:(q + 1) * MMW],
                                xT[:, ic * 128:(ic + 1) * 128],
                                m2yT[:, j0:j0 + MMW])
                        u = upool.tile([128, gw], bf16, tag="u", name="u")
                        nc.vector.tensor_tensor(
                            u[:, :], psum[:, :], y2bc[:, sl], AluOp.add)
                        k = ic * n_jg + jg
                        nc.vector.tensor_reduce(
                            rowR[:, k:k + 1], u[:, :],
                            mybir.AxisListType.X, AluOp.min)
                        nc.vector.scalar_tensor_tensor(
                            colB[:, sl], u[:, :], x2c[:, ic:ic + 1],
                            colB[:, sl], AluOp.add, AluOp.min)

            if reps > 1:
                with tc.For_i(0, reps, 1,
                              hint_engines=(mybir.EngineType.PE,
                                            mybir.EngineType.DVE)):
                    emit_body()
            else:
                emit_body()

            for c0 in range(0, nj, ck):
                nc.sync.dma_start(out=colB_d[:, c0:c0 + ck], in_=colB[:, c0:c0 + ck])
            nc.sync.dma_start(out=rowR_d[:, :], in_=rowR[:, :])

    nc.compile()
    return nc


def build_ttr(ni=NI, nj=NJ, gw=GW, reps=1, use_rank1=True, use_ttr=True):
    """One-orientation scheme, 2 DVE ops per [128 x gw] psum group:

      PE:  psum = -2*z + x2_i   (4 data MMs + 4 rank-1 MMs per group;
           rank-1: stationary row0 = x2 chunk, moving row0 = ones)
      DVE: tensor_tensor_reduce: u_bf16 = psum + y2bc  (full P),
           rowR[:,k] = min_j u   -- fused conversion + row-min (1x pass)
      DVE: tensor_tensor:        colB = min(colB, u)   (bf16 2x pass)

    Host: row mins = min over jg of rowR; col mins = min over lanes/cores
    of colB; clip(0,100) commutes with min; then means.
    """
    n_ic = ni // 128
    n_jg = nj // gw

    nc = bacc.Bacc("TRN2", target_bir_lowering=False, debug=False,
                   enable_asserts=False, num_devices=NCORES)
    f32 = mybir.dt.float32
    bf16 = mybir.dt.bfloat16

    xT_d = nc.dram_tensor("xT", [128, ni], bf16, kind="ExternalInput")
    m2yT_d = nc.dram_tensor("m2yT", [128, nj], bf16, kind="ExternalInput")
    if use_rank1:
        x2r_d = nc.dram_tensor("x2r", [128, n_ic * 128], bf16,
                               kind="ExternalInput")
        ones_d = nc.dram_tensor("ones", [128, MMW], bf16, kind="ExternalInput")
    else:
        x2c_d = nc.dram_tensor("x2c", [128, n_ic], f32, kind="ExternalInput")
    y2bc_d = nc.dram_tensor("y2bc", [128, nj], bf16, kind="ExternalInput")
    colB_d = nc.dram_tensor("colB", [128, nj], bf16, kind="ExternalOutput")
    rowR_d = nc.dram_tensor("rowR", [128, n_ic * n_jg], f32, kind="ExternalOutput")

    with tile.TileContext(nc) as tc:
        with (
            tc.tile_pool(name="persist", bufs=1) as persist,
            tc.tile_pool(name="psum", bufs=2, space="PSUM") as psum_pool,
            tc.tile_pool(name="u", bufs=3) as upool,
        ):
            xT = persist.tile([128, ni], bf16, name="xT")
            m2yT = persist.tile([128, nj], bf16, name="m2yT")
            if use_rank1:
                x2r = persist.tile([128, n_ic * 128], bf16, name="x2r")
                ones = persist.tile([128, MMW], bf16, name="ones")
            else:
                x2c = persist.tile([128, n_ic], f32, name="x2c")
            y2bc = persist.tile([128, nj], bf16, name="y2bc")
            colB = persist.tile([128, nj], bf16, name="colB")
            rowR = persist.tile([128, n_ic * n_jg], f32, name="rowR")

            nc.sync.dma_start(out=xT[:, :], in_=xT_d[:, :])
            if use_rank1:
                nc.sync.dma_start(out=x2r[:, :], in_=x2r_d[:, :])
                nc.sync.dma_start(out=ones[:, :], in_=ones_d[:, :])
            else:
                nc.sync.dma_start(out=x2c[:, :], in_=x2c_d[:, :])
            ck = min(2048, nj)
            for c0 in range(0, nj, ck):
                nc.sync.dma_start(out=m2yT[:, c0:c0 + ck], in_=m2yT_d[:, c0:c0 + ck])
                nc.sync.dma_start(out=y2bc[:, c0:c0 + ck], in_=y2bc_d[:, c0:c0 + ck])
            nc.vector.memset(colB[:, :], BIG)

            def emit_body():
                for ic in range(n_ic):
                    for jg in range(n_jg):
                        sl = slice(jg * gw, (jg + 1) * gw)
                        psum = psum_pool.tile([128, gw], f32, tag="ps", name="ps")
                        for q in range(gw // MMW):
                            j0 = jg * gw + q * MMW
                            nc.tensor.matmul(
                                psum[:, q * MMW:(q + 1) * MMW],
                                xT[:, ic * 128:(ic + 1) * 128],
                                m2yT[:, j0:j0 + MMW],
                                start=True, stop=not use_rank1)
                        if use_rank1:
                            for q in range(gw // MMW):
                                nc.tensor.matmul(
                                    psum[:, q * MMW:(q + 1) * MMW],
                                    x2r[:, ic * 128:(ic + 1) * 128],
                                    ones[:, :],
                                    start=False, stop=True)
                        u = upool.tile([128, gw], bf16, tag="u", name="u")
                        k = ic * n_jg + jg
                        if use_ttr:
                            nc.vector.tensor_tensor_reduce(
                                u[:, :], psum[:, :], y2bc[:, sl],
                                1.0, BIG, AluOp.add, AluOp.min,
                                rowR[:, k:k + 1])
                        else:
                            nc.vector.tensor_tensor(
                                u[:, :], psum[:, :], y2bc[:, sl], AluOp.add)
                            nc.vector.tensor_reduce(
                                rowR[:, k:k + 1], u[:, :],
                                mybir.AxisListType.X, AluOp.min)
                        if use_rank1:
                            nc.vector.tensor_tensor(
                                colB[:, sl], u[:, :], colB[:, sl], AluOp.min)
                        else:
                            nc.vector.scalar_tensor_tensor(
                                colB[:, sl], u[:, :], x2c[:, ic:ic + 1],
                                colB[:, sl], AluOp.add, AluOp.min)

            if reps > 1:
                with tc.For_i(0, reps, 1,
                              hint_engines=(mybir.EngineType.PE,
                                            mybir.EngineType.DVE)):
                    emit_body()
            else:
                emit_body()

            for c0 in range(0, nj, ck):
                nc.sync.dma_start(out=colB_d[:, c0:c0 + ck], in_=colB[:, c0:c0 + ck])
            nc.sync.dma_start(out=rowR_d[:, :], in_=rowR[:, :])

    nc.compile()
    return nc


TW = 512  # row-min tree stop width (host finishes the reduction)


def build_v3(ni=NI, nj=NJ, gw=GW, reps=1):
    """Measured-rate design (TTR unusable on this HW):

      PE:  psum = -2z + x2_i + y2_j   (4 data MMs + 4 rank-2 MMs;
           rank-2 stationary row0 = x2 chunk, row1 = ones;
           rank-2 moving row0 = ones, row1 = y2)
      ACT: u = bf16(psum)        1x @1.2GHz, own engine   (~1966ns/group)
      DVE: colB = min(colB, u)   bf16 SBUF 2x             (~1226ns/group)
      DVE: row-min tree of TT-mins at 2x down to TW, then one segmented
           tensor_reduce [128, n_ic, TW] -> [128, n_ic] at the end.

    Host: rowR/colB hold full-P mins; clip+mean only.
    """
    n_ic = ni // 128
    n_jg = nj // gw

    nc = bacc.Bacc("TRN2", target_bir_lowering=False, debug=False,
                   enable_asserts=False, num_devices=NCORES)
    f32 = mybir.dt.float32
    bf16 = mybir.dt.bfloat16

    xT_d = nc.dram_tensor("xT", [128, ni], bf16, kind="ExternalInput")
    m2yT_d = nc.dram_tensor("m2yT", [128, nj], bf16, kind="ExternalInput")
    r2s_d = nc.dram_tensor("r2s", [128, n_ic * 128], bf16, kind="ExternalInput")
    r2m_d = nc.dram_tensor("r2m", [128, nj], bf16, kind="ExternalInput")
    colB_d = nc.dram_tensor("colB", [128, nj], bf16, kind="ExternalOutput")
    rowA_d = nc.dram_tensor("rowA", [128, n_ic * TW], bf16, kind="ExternalOutput")

    with tile.TileContext(nc) as tc:
        with (
            tc.tile_pool(name="persist", bufs=1) as persist,
            tc.tile_pool(name="psum", bufs=2, space="PSUM") as psum_pool,
            tc.tile_pool(name="u", bufs=4) as upool,
            tc.tile_pool(name="tree", bufs=2) as tpool,
        ):
            xT = persist.tile([128, ni], bf16, name="xT")
            m2yT = persist.tile([128, nj], bf16, name="m2yT")
            r2s = persist.tile([128, n_ic * 128], bf16, name="r2s")
            r2m = persist.tile([128, nj], bf16, name="r2m")
            colB = persist.tile([128, nj], bf16, name="colB")
            rowacc = persist.tile([128, n_ic * TW], bf16, name="rowacc")

            # DMA priority order = first-use order: group (0,0) inputs,
            # then remaining j-chunks (used within ~4us), then xT/r2s tails
            fk = min(512, ni)
            nc.sync.dma_start(out=xT[:, :fk], in_=xT_d[:, :fk])
            nc.sync.dma_start(out=r2s[:, :fk], in_=r2s_d[:, :fk])
            fj = min(gw, nj)
            nc.sync.dma_start(out=m2yT[:, :fj], in_=m2yT_d[:, :fj])
            nc.sync.dma_start(out=r2m[:, :fj], in_=r2m_d[:, :fj])
            for c0 in range(fj, nj, gw):
                nc.sync.dma_start(out=m2yT[:, c0:c0 + gw], in_=m2yT_d[:, c0:c0 + gw])
                nc.sync.dma_start(out=r2m[:, c0:c0 + gw], in_=r2m_d[:, c0:c0 + gw])
            for c0 in range(fk, ni, 512):
                nc.sync.dma_start(out=xT[:, c0:c0 + 512], in_=xT_d[:, c0:c0 + 512])
                nc.sync.dma_start(out=r2s[:, c0:c0 + 512], in_=r2s_d[:, c0:c0 + 512])

            def emit_body():
                for ic in range(n_ic):
                    st = slice(ic * 128, (ic + 1) * 128)
                    us = []
                    for jg in range(n_jg):
                        sl = slice(jg * gw, (jg + 1) * gw)
                        psum = psum_pool.tile([128, gw], f32, tag="ps", name="ps")
                        for q in range(gw // MMW):
                            j0 = jg * gw + q * MMW
                            nc.tensor.matmul(
                                psum[:, q * MMW:(q + 1) * MMW],
                                xT[:, st], m2yT[:, j0:j0 + MMW],
                                start=True, stop=False)
                        for q in range(gw // MMW):
                            j0 = jg * gw + q * MMW
                            nc.tensor.matmul(
                                psum[:, q * MMW:(q + 1) * MMW],
                                r2s[:, st], r2m[:, j0:j0 + MMW],
                                start=False, stop=True)
                        u = upool.tile([128, gw], bf16, tag="u", name="u")
                        nc.scalar.activation(u[:, :], psum[:, :],
                                             ActFn.Identity, bias=0.0,
                                             scale=1.0)
                        if ic == 0:
                            nc.vector.tensor_copy(colB[:, sl], u[:, :])
                        else:
                            nc.vector.tensor_tensor(
                                colB[:, sl], u[:, :], colB[:, sl], AluOp.min)
                        us.append(u)
                    # row-min tree: gw*n_jg -> TW in bf16 TT-mins (2x)
                    h = gw // 2
                    a = tpool.tile([128, h], bf16, tag="a", name="a")
                    nc.vector.tensor_tensor(
                        a[:, :], us[0][:, :h], us[0][:, h:], AluOp.min)
                    for u in us[1:]:
                        nc.vector.tensor_tensor(
                            a[:, :], a[:, :], u[:, :h], AluOp.min)
                        nc.vector.tensor_tensor(
                            a[:, :], a[:, :], u[:, h:], AluOp.min)
                    w = h
                    while w > TW * 2:
                        w //= 2
                        nc.vector.tensor_tensor(
                            a[:, :w], a[:, :w], a[:, w:2 * w], AluOp.min)
                    rsl = slice(ic * TW, (ic + 1) * TW)
                    nc.vector.tensor_tensor(
                        rowacc[:, rsl], a[:, :TW], a[:, TW:2 * TW], AluOp.min)
                    nc.sync.dma_start(out=rowA_d[:, rsl], in_=rowacc[:, rsl])

            if reps > 1:
                with tc.For_i(0, reps, 1,
                              hint_engines=(mybir.EngineType.PE,
                                            mybir.EngineType.DVE,
                                            mybir.EngineType.Activation)):
                    emit_body()
            else:
                emit_body()

            for c0 in range(0, nj, gw):
                nc.sync.dma_start(out=colB_d[:, c0:c0 + gw], in_=colB[:, c0:c0 + gw])

    nc.compile()
    return nc


def build_pf(ni, nj, gw, reps):
    """Two-orientation scheme with DVE/ACT split (fallback / A-B testing)."""
    n_ic = ni // 128
    n_jc = nj // 128
    pat1 = PAT1[:n_ic]
    pat2 = PAT2[:n_jc]
    paths = set(pat1) | set(pat2)

    nc = bacc.Bacc("TRN2", target_bir_lowering=False, debug=False,
                   enable_asserts=False, num_devices=NCORES)
    f32 = mybir.dt.float32
    bf16 = mybir.dt.bfloat16

    xT_d = nc.dram_tensor("xT", [128, ni], bf16, kind="ExternalInput")
    m2yT_d = nc.dram_tensor("m2yT", [128, nj], bf16, kind="ExternalInput")
    x2c_d = nc.dram_tensor("x2c", [128, n_ic], f32, kind="ExternalInput")
    y2c_d = nc.dram_tensor("y2c", [128, n_jc], f32, kind="ExternalInput")
    col_d, row_d = {}, {}
    for p in sorted(paths):
        dt = f32 if p == 'D' else bf16
        col_d[p] = nc.dram_tensor("col" + p, [128, nj], dt, kind="ExternalOutput")
        row_d[p] = nc.dram_tensor("row" + p, [128, ni], dt, kind="ExternalOutput")

    with tile.TileContext(nc) as tc:
        with (
            tc.tile_pool(name="persist", bufs=1) as persist,
            tc.tile_pool(name="psum", bufs=4, space="PSUM") as psum_pool,
            tc.tile_pool(name="u", bufs=6) as upool,
        ):
            xT = persist.tile([128, ni], bf16, name="xT")
            m2yT = persist.tile([128, nj], bf16, name="m2yT")
            x2c = persist.tile([128, n_ic], f32, name="x2c")
            y2c = persist.tile([128, n_jc], f32, name="y2c")
            col_s = {p: persist.tile([128, nj], f32 if p == 'D' else bf16,
                                     name="col" + p, tag="col" + p)
                     for p in sorted(paths)}
            row_s = {p: persist.tile([128, ni], f32 if p == 'D' else bf16,
                                     name="row" + p, tag="row" + p)
                     for p in sorted(paths)}

            ck = min(1024, ni, nj)
            for c0 in range(0, ni, ck):
                nc.sync.dma_start(out=xT[:, c0:c0 + ck], in_=xT_d[:, c0:c0 + ck])
            for c0 in range(0, nj, ck):
                nc.sync.dma_start(out=m2yT[:, c0:c0 + ck], in_=m2yT_d[:, c0:c0 + ck])
            nc.sync.dma_start(out=x2c[:, :], in_=x2c_d[:, :])
            nc.sync.dma_start(out=y2c[:, :], in_=y2c_d[:, :])

            def consume(path, psum, bias, accs, sl, first):
                acc = accs[path]
                if path == 'D':
                    if first:
                        nc.vector.tensor_scalar(
                            acc[:, sl], psum[:, :], bias, None, AluOp.add)
                    else:
                        nc.vector.scalar_tensor_tensor(
                            acc[:, sl], psum[:, :], bias, acc[:, sl],
                            AluOp.add, AluOp.min)
                    return
                u = upool.tile([128, psum.shape[1]], bf16, name="u", tag="u")
                nc.scalar.activation(u[:, :], psum[:, :], ActFn.Identity,
                                     bias=bias, scale=1.0)
                if first:
                    nc.vector.tensor_copy(acc[:, sl], u[:, :])
                else:
                    nc.vector.tensor_tensor(acc[:, sl], u[:, :], acc[:, sl],
                                            AluOp.min)

            def emit_body():
                for jg in range(nj // gw):
                    sl = slice(jg * gw, (jg + 1) * gw)
                    seen = set()
                    for ic in range(n_ic):
                        path = pat1[ic]
                        psum = psum_pool.tile([128, gw], f32, tag="ps", name="ps")
                        for q in range(gw // MMW):
                            j0 = jg * gw + q * MMW
                            nc.tensor.matmul(
                                psum[:, q * MMW:(q + 1) * MMW],
                                xT[:, ic * 128:(ic + 1) * 128],
                                m2yT[:, j0:j0 + MMW])
                        consume(path, psum, x2c[:, ic:ic + 1], col_s, sl,
                                path not in seen)
                        seen.add(path)
                gw2 = min(gw, ni)
                for ig in range(ni // gw2):
                    sl = slice(ig * gw2, (ig + 1) * gw2)
                    seen = set()
                    for jc in range(n_jc):
                        path = pat2[jc]
                        psum = psum_pool.tile([128, gw2], f32, tag="ps", name="ps")
                        for q in range(gw2 // MMW):
                            i0 = ig * gw2 + q * MMW
                            nc.tensor.matmul(
                                psum[:, q * MMW:(q + 1) * MMW],
                                m2yT[:, jc * 128:(jc + 1) * 128],
                                xT[:, i0:i0 + MMW])
                        consume(path, psum, y2c[:, jc:jc + 1], row_s, sl,
                                path not in seen)
                        seen.add(path)

            if reps > 1:
                with tc.For_i(0, reps, 1,
                              hint_engines=(mybir.EngineType.PE,
                                            mybir.EngineType.DVE,
                                            mybir.EngineType.Activation)):
                    emit_body()
            else:
                emit_body()

            for p in sorted(paths):
                nc.sync.dma_start(out=col_d[p][:, :], in_=col_s[p][:, :])
                nc.sync.dma_start(out=row_d[p][:, :], in_=row_s[p][:, :])

    nc.compile()
    return nc


def host_prep(x, y, scheme="hybrid"):
    """Per-core input maps. Core c: batch c//2, i-half c%2."""
    x = np.ascontiguousarray(np.asarray(x, F32))
    y = np.ascontiguousarray(np.asarray(y, F32))
    x16 = x.astype(BF16)
    y16 = y.astype(BF16)
    m2y16 = (y16.astype(F32) * -2.0).astype(BF16)          # exact in bf16
    x2 = (x16.astype(F32) ** 2).sum(-1)                    # [B, N]
    y2 = (y16.astype(F32) ** 2).sum(-1)
    in_maps = []
    for c in range(NCORES):
        b, h = divmod(c, 2)
        i0 = h * NI
        m = {
            "xT": np.ascontiguousarray(x16[b, i0:i0 + NI, :].T),
            "m2yT": np.ascontiguousarray(m2y16[b].T),
        }
        if scheme == "v3":
            r2s = np.zeros((128, NI), BF16)
            r2s[0, :] = x2[b, i0:i0 + NI].astype(BF16)
            r2s[1, :] = 1.0
            m["r2s"] = r2s
            r2m = np.zeros((128, N), BF16)
            r2m[0, :] = 1.0
            r2m[1, :] = y2[b].astype(BF16)
            m["r2m"] = r2m
        elif scheme.startswith("ttr"):
            if "nor1" in scheme:
                m["x2c"] = np.ascontiguousarray(
                    x2[b, i0:i0 + NI].reshape(NI // 128, 128).T)
            else:
                x2r = np.zeros((128, NI), BF16)
                x2r[0, :] = x2[b, i0:i0 + NI].astype(BF16)
                m["x2r"] = x2r
                ones = np.zeros((128, MMW), BF16)
                ones[0, :] = 1.0
                m["ones"] = ones
            m["y2bc"] = np.ascontiguousarray(
                np.broadcast_to(y2[b].astype(BF16), (128, N)))
        elif scheme == "hybrid":
            m["x2c"] = np.ascontiguousarray(
                x2[b, i0:i0 + NI].reshape(NI // 128, 128).T)
            m["y2bc"] = np.ascontiguousarray(
                np.broadcast_to(y2[b].astype(BF16), (128, N)))
        else:
            m["x2c"] = np.ascontiguousarray(
                x2[b, i0:i0 + NI].reshape(NI // 128, 128).T)
            m["y2c"] = np.ascontiguousarray(y2[b].reshape(N // 128, 128).T)
        in_maps.append(m)
    return in_maps, x2, y2


def combine(results, x2, y2, scheme="hybrid"):
    col_mins = np.empty((B, N), F32)
    row_mins = np.empty((B, N), F32)
    for b in range(B):
        cores = [results[2 * b], results[2 * b + 1]]
        if scheme == "v3":
            col = np.minimum.reduce([r["colB"].astype(F32).min(0) for r in cores])
            col_mins[b] = np.clip(col, 0.0, 100.0)
            for h, r in enumerate(cores):
                ra = r["rowA"].astype(F32)             # [128, n_ic*TW]
                row = ra.reshape(128, NI // 128, TW).min(axis=2).T.reshape(-1)
                i0 = h * NI
                row_mins[b, i0:i0 + NI] = np.clip(row, 0.0, 100.0)
        elif scheme.startswith("ttr"):
            # colB holds full P mins; clip commutes with min
            col = np.minimum.reduce([r["colB"].astype(F32).min(0) for r in cores])
            col_mins[b] = np.clip(col, 0.0, 100.0)
            for h, r in enumerate(cores):
                rr = r["rowR"]                         # [128, n_ic*n_jg]
                n_jg = N // GW
                rr = rr.reshape(128, NI // 128, n_jg).min(axis=2)
                row = rr.T.reshape(-1)                 # [NI], i = ic*128 + lane
                i0 = h * NI
                if "nor1" in scheme:                   # psum lacked x2
                    row = row + x2[b, i0:i0 + NI]
                row_mins[b, i0:i0 + NI] = np.clip(row, 0.0, 100.0)
        elif scheme == "hybrid":
            col = np.minimum.reduce([r["colB"].astype(F32).min(0) for r in cores])
            col_mins[b] = np.clip(col, 0.0, 100.0)
            for h, r in enumerate(cores):
                rr = r["rowR"]                         # [128, n_ic*n_jg]
                n_jg = N // GW
                rr = rr.reshape(128, NI // 128, n_jg).min(axis=2)
                row = rr.T.reshape(-1)                 # [NI], i = ic*128 + lane
                i0 = h * NI
                row_mins[b, i0:i0 + NI] = np.clip(
                    row + x2[b, i0:i0 + NI], 0.0, 100.0)
        else:
            col = np.minimum.reduce([
                np.minimum.reduce([r[k].astype(F32).min(0)
                                   for k in r if k.startswith("col")])
                for r in cores])
            col_mins[b] = np.clip(col + y2[b], 0.0, 100.0)
            for h, r in enumerate(cores):
                row = np.minimum.reduce([r[k].astype(F32).min(0)
                                         for k in r if k.startswith("row")])
                i0 = h * NI
                row_mins[b, i0:i0 + NI] = np.clip(
                    row + x2[b, i0:i0 + NI], 0.0, 100.0)
    out = (col_mins.mean(dtype=np.float64) + row_mins.mean(dtype=np.float64)) / B
    return np.asarray(out, dtype=F32)


_CACHE = {}
TRACE = False
LAST_RESULTS = None
SCHEME = "ttr"


def kernel(corr_pred, corr_target):
    global LAST_RESULTS
    key = ("nc", SCHEME)
    if key not in _CACHE:
        _CACHE[key] = build(scheme=SCHEME)
    nc = _CACHE[key]
    in_maps, x2, y2 = host_prep(corr_pred, corr_target, scheme=SCHEME)
    res = run_bass_kernel_spmd(nc, in_maps, core_ids=list(range(NCORES)),
                               trace=TRACE)
    LAST_RESULTS = res
    return combine(res.results, x2, y2, scheme=SCHEME)

